# revision 16
# baseline (speedup 1.0000x reference)
"""Trainium2 Bass kernel for AxonalConnections message passing.

Computes out[b, t] = sum_s spikes[b, s] * adjacency[t, s]
  spikes_A: [8, 128, 128] f32  -> flat [B=8, S=16384]
  adjacency: [16384, 16384] f32 (1 GiB -- the memory-bound stream)
  out: [8, 128, 128] f32

Strategy (8 NeuronCores, SPMD):
  - Shard adjacency row-wise over the target dim T: core m owns rows
    [m*2048, (m+1)*2048).  Each core computes its own output column
    block; no collectives.
  - The PE contracts over the partition dim, so the big operand must sit
    in SBUF with S on partitions.  We pre-transpose each core's block on
    the host (adjacency[t0:t1, :].T, shape [S, 2048]) so device DMA is
    large and contiguous.  The memory roofline (stream 1 GiB over 8 NCs'
    HBM) is unchanged by host-side layout.
  - fp32 matmul runs at 4 cycles/row on the PE (dual half-rate passes),
    which would make the PE the bottleneck (>520 us).  Instead each fp32
    value is split on the host into two fp16 halves (hi + lo; the split
    is exact to 2^-22 relative).  Total DMA bytes are unchanged
    (2 x 2 B), but the PE streams fp16 rows at 1 cycle/row: per s-stripe
    the stationary is [xh | xl] (16 cols) and two moving passes (a_hi,
    a_lo) accumulate all four cross terms into PSUM rows 0-7 (xh*a) and
    8-15 (xl*a).  A final DVE add folds the halves.  Result matches fp32
    to ~1e-6 relative (verified vs the fp32 reference).
  - Per core: 16 slabs of [128, 8 stripes, 2, 2048] fp16 (8 MiB each,
    contiguous), double buffered; 8 matmuls (N=512) per stripe into 4
    PSUM banks.
"""

import sys

if "/opt/trn_rl_repo" not in sys.path:
    sys.path.insert(0, "/opt/trn_rl_repo")

from concurrent.futures import ThreadPoolExecutor

import numpy as np

N_CORES = 8
B = 8
S = 16384            # source neurons (contraction dim)
T = 16384            # target neurons
P = 128              # partitions
TBLK = T // N_CORES  # 2048 targets per core
S_TILES = S // P     # 128 stripes of the contraction dim
G = 8                # s-stripes per DMA slab (8 MiB)
NGRP = S_TILES // G  # 16 slabs
TCH = 512            # psum chunk (one bank, fp32)
NCH = TBLK // TCH    # 4

_prog_cache = {}


def _build_program():
    import concourse.bacc as bacc
    import concourse.tile as tile
    from concourse import bass, mybir

    f16 = mybir.dt.float16
    f32 = mybir.dt.float32

    nc = bacc.Bacc("TRN2", target_bir_lowering=False, debug=False)
    adjt2 = nc.dram_tensor("adjt2", [S, 2, TBLK], f16, kind="ExternalInput").ap()
    xt = nc.dram_tensor("xt", [P, S_TILES * 2 * B], f16, kind="ExternalInput").ap()
    # rows 0-7: xh*(ah+al); rows 8-15: xl*(ah+al); folded on the host
    y2 = nc.dram_tensor("y2", [2 * B, TBLK], f32, kind="ExternalOutput").ap()

    with tile.TileContext(nc) as tc:
        with (
            tc.tile_pool(name="adj", bufs=3) as adj_pool,
            tc.tile_pool(name="misc", bufs=1) as misc_pool,
            tc.tile_pool(name="psum", bufs=1, space=bass.MemorySpace.PSUM) as psum_pool,
        ):
            xt_sb = misc_pool.tile([P, S_TILES * 2 * B], f16)
            nc.sync.dma_start(xt_sb[:], xt[:])
            y_sb = misc_pool.tile([2 * B, TBLK], f32)
            psums = [
                psum_pool.tile([2 * B, TCH], f32, name=f"psum{j}") for j in range(NCH)
            ]

            # [S, 2, TBLK] -> [P, S_TILES, 2, TBLK]: stripe i on partition p
            adjt2_r = adjt2.rearrange("(i p) h t -> p i h t", p=P)
            slabs = [G] * (S_TILES // G)
            off = 0
            for si, sz in enumerate(slabs):
                at = adj_pool.tile([P, sz, 2, TBLK], f16, name="at", tag="at")
                if si >= len(slabs) - 2:
                    # final slab: per-stripe sub-DMAs into the same slot, so the
                    # PE tail after the stream ends is one stripe, not eight.
                    # (Extra dma_start boundaries only degrade the stream when
                    # more data queues behind them -- harmless at the end.)
                    for g in range(sz):
                        nc.sync.dma_start(
                            at[:, g : g + 1], adjt2_r[:, off + g : off + g + 1]
                        )
                else:
                    nc.sync.dma_start(at[:], adjt2_r[:, off : off + sz])
                for g in range(sz):
                    i = off + g
                    lhsT = xt_sb[:, i * 2 * B : (i + 1) * 2 * B]  # [xh | xl]
                    for j in range(NCH):
                        for h in range(2):  # moving pass over a_hi then a_lo
                            nc.tensor.matmul(
                                psums[j][:],
                                lhsT,
                                at[:, g, h, j * TCH : (j + 1) * TCH],
                                start=(i == 0 and h == 0),
                                stop=(i == S_TILES - 1 and h == 1),
                            )
                off += sz
            assert off == S_TILES
            for j in range(NCH):
                nc.vector.tensor_copy(y_sb[:, j * TCH : (j + 1) * TCH], psums[j][:])
            nc.sync.dma_start(y2[:], y_sb[:])

    nc.compile()
    return nc


def _get_program():
    if "p" not in _prog_cache:
        _prog_cache["p"] = _build_program()
    return _prog_cache["p"]


def _split16(a32):
    hi = a32.astype(np.float16)
    lo = (a32 - hi.astype(np.float32)).astype(np.float16)
    return hi, lo


def _host_prep(spikes_A, adjacency):
    flat = np.ascontiguousarray(np.asarray(spikes_A, dtype=np.float32)).reshape(B, S)
    xh, xl = _split16(flat)
    # xt[p, i*16 + h*8 + b] = x_half[h][b, i*128 + p]
    arr = np.stack([xh.reshape(B, S_TILES, P), xl.reshape(B, S_TILES, P)], axis=0)
    xt_host = np.ascontiguousarray(
        arr.transpose(3, 2, 0, 1).reshape(P, S_TILES * 2 * B)
    )
    adj = np.asarray(adjacency, dtype=np.float32)

    def prep_core(m):
        blkT = np.ascontiguousarray(adj[m * TBLK : (m + 1) * TBLK, :].T)  # [S, TBLK]
        ah, al = _split16(blkT)
        adjt2_m = np.ascontiguousarray(np.stack([ah, al], axis=1))  # [S, 2, TBLK]
        return {"adjt2": adjt2_m, "xt": xt_host}

    with ThreadPoolExecutor(max_workers=N_CORES) as ex:
        in_maps = list(ex.map(prep_core, range(N_CORES)))
    return in_maps


def run(spikes_A, adjacency, trace=False):
    """Run on hardware; returns (out [8,128,128] f32, BassKernelResults)."""
    from concourse.bass_utils import run_bass_kernel_spmd

    nc = _get_program()
    in_maps = _host_prep(spikes_A, adjacency)
    res = run_bass_kernel_spmd(nc, in_maps, core_ids=list(range(N_CORES)), trace=trace)
    out = np.concatenate(
        [res.results[m]["y2"][0:B] + res.results[m]["y2"][B : 2 * B]
         for m in range(N_CORES)],
        axis=1,
    )
    return out.reshape(B, 128, 128), res


def kernel(spikes_A, adjacency):
    out, _ = run(spikes_A, adjacency, trace=False)
    return out
